# revision 1
# baseline (speedup 1.0000x reference)
"""BertSum attention kernel v8 - v7 + finer proj interleave, dual-engine staging.

Sharding: core c = (batch b = c//2, head-block hb = c%2). Each core
computes heads hb*8..hb*8+8 for ALL 2048 queries of its batch, plus the
partial output projection for its 512 channels; the host sums the two
partial outputs per batch. This removes the duplicated K/V projections
of query-sharding (each core projects only its own 8 heads' K/Q/V).

Same engine assignments as v4: bf16 matmuls, SBUF-resident mask (8MB,
loaded once), [128,1024] exp/mask tiles, fused v-copy, reciprocal +
GpSimd partition_broadcast normalization, bf16 output.
"""

import numpy as np
from contextlib import ExitStack

import ml_dtypes

import concourse.bass as bass
import concourse.mybir as mybir
from concourse import bacc
from concourse.tile import TileContext
from concourse.bass_utils import run_bass_kernel_spmd

F32 = mybir.dt.float32
BF16 = mybir.dt.bfloat16
AF = mybir.ActivationFunctionType
ALU = mybir.AluOpType

B, S, D = 4, 2048, 1024
H, DH = 16, 64
SQ = 1024          # attention q-chunk width (half of S)
NPL = 4            # local head-pairs per core (8 heads)

_CACHE = {}


def _build(reps=1):
    nc = bacc.Bacc("TRN2", target_bir_lowering=False)

    d16d = nc.declare_dram_parameter("d16", [128, 8, S], BF16, isOutput=False)
    maskT = nc.declare_dram_parameter("maskT", [S, S], BF16, isOutput=False)
    wq16 = nc.declare_dram_parameter("wq16", [128, NPL, 8, 128], BF16,
                                     isOutput=False)
    wk16 = nc.declare_dram_parameter("wk16", [128, NPL, 8, 128], BF16,
                                     isOutput=False)
    wv16 = nc.declare_dram_parameter("wv16", [128, NPL // 2, 8, 256], BF16,
                                     isOutput=False)
    woT = nc.declare_dram_parameter("woT", [NPL * 128, D], BF16,
                                    isOutput=False)
    bq2 = nc.declare_dram_parameter("bq2", [128, NPL], F32, isOutput=False)
    bk2 = nc.declare_dram_parameter("bk2", [128, NPL], F32, isOutput=False)
    out = nc.declare_dram_parameter("out", [S, D], BF16, isOutput=True)

    with ExitStack() as ctx:
        ctx.enter_context(nc.allow_low_precision(
            reason="bf16 matmul operands; accumulation stays f32"))
        tc = ctx.enter_context(TileContext(nc))
        const = ctx.enter_context(tc.tile_pool(name="const", bufs=1))
        dpool = ctx.enter_context(tc.tile_pool(name="data", bufs=1))
        ctxp = ctx.enter_context(tc.tile_pool(name="ctxT", bufs=1))

        bqsb = const.tile([128, NPL], F32)
        nc.sync.dma_start(out=bqsb, in_=bq2[:, :])
        bksb = const.tile([128, NPL], F32)
        nc.sync.dma_start(out=bksb, in_=bk2[:, :])

        d16 = dpool.tile([128, 8, S], BF16, name="d16sb")
        msb = dpool.tile([128, 16, S], BF16, name="msb")

        def emit_data_dmas():
            # after pair-0 weight DMAs so they don't block the first matmuls
            for i in range(8):
                nc.sync.dma_start(out=d16[:, i, :], in_=d16d[:, i, :])
            # mask rides the second HWDGE queue (Activation)
            for i in range(16):
                nc.scalar.dma_start(out=msb[:, i, :],
                                    in_=maskT[i * 128:(i + 1) * 128, :])

        for rep in range(reps):
            ctxT = [ctxp.tile([128, S], BF16, tag=f"ctx{p}",
                              name=f"ctxT{rep}_{p}")
                    for p in range(NPL)]

            rctx = ExitStack()
            wop = rctx.enter_context(tc.tile_pool(name="wo", bufs=1))
            with ExitStack() as actx:
                wkp = actx.enter_context(tc.tile_pool(name="wk", bufs=2))
                wqp = actx.enter_context(tc.tile_pool(name="wq", bufs=2))
                wvp = actx.enter_context(tc.tile_pool(name="wv", bufs=1))
                kqp = actx.enter_context(tc.tile_pool(name="kqp", bufs=1))
                vpool = actx.enter_context(tc.tile_pool(name="vp", bufs=2))
                epool = actx.enter_context(tc.tile_pool(name="exp", bufs=4))
                rpool = actx.enter_context(tc.tile_pool(name="rec", bufs=2))
                bcp = actx.enter_context(tc.tile_pool(name="bcp", bufs=2))
                ssp = actx.enter_context(
                    tc.tile_pool(name="ssp", bufs=2, space="PSUM"))
                cpsp = actx.enter_context(
                    tc.tile_pool(name="cpsp", bufs=2, space="PSUM"))

                k_tiles, q_tiles = [], []
                for j in range(2):
                    k_tiles.append(kqp.tile([128, S], BF16, tag=f"k{j}",
                                            name=f"kt{rep}_{j}"))
                    q_tiles.append(kqp.tile([128, S], BF16, tag=f"q{j}",
                                            name=f"qt{rep}_{j}"))

                v_tiles = {}

                def emit_proj(p):
                    """Returns (dma_thunk, [4 compute thunks])."""
                    kt = k_tiles[p % 2]
                    qt = q_tiles[p % 2]
                    st_ = {}

                    def dmas():
                        st_["wk"] = wkp.tile([128, 8, 128], BF16, tag="wk",
                                             name="wk_sb")
                        nc.sync.dma_start(out=st_["wk"], in_=wk16[:, p, :, :])
                        st_["wq"] = wqp.tile([128, 8, 128], BF16, tag="wq",
                                             name="wq_sb")
                        nc.sync.dma_start(out=st_["wq"], in_=wq16[:, p, :, :])
                        if p % 2 == 0:
                            st_["wv"] = wvp.tile([128, 8, 256], BF16,
                                                 tag="wv", name="wv_sb")
                            nc.sync.dma_start(out=st_["wv"],
                                              in_=wv16[:, p // 2, :, :])

                    def c_k(scs):
                        for sc in scs:
                            ps = ssp.tile([128, SQ], F32, tag="ss",
                                          name="ps_k")
                            for i in range(8):
                                nc.tensor.matmul(
                                    ps[:, 0:512], st_["wk"][:, i, :],
                                    d16[:, i, sc * 512:(sc + 1) * 512],
                                    start=(i == 0), stop=(i == 7))
                            nc.vector.tensor_scalar_add(
                                kt[:, sc * 512:(sc + 1) * 512], ps[:, 0:512],
                                bksb[:, p:p + 1])

                    def c_q(scs):
                        for sc in scs:
                            ps = ssp.tile([128, SQ], F32, tag="ss",
                                          name="ps_q")
                            for i in range(8):
                                nc.tensor.matmul(
                                    ps[:, 0:512], st_["wq"][:, i, :],
                                    d16[:, i, sc * 512:(sc + 1) * 512],
                                    start=(i == 0), stop=(i == 7))
                            nc.vector.tensor_scalar_add(
                                qt[:, sc * 512:(sc + 1) * 512], ps[:, 0:512],
                                bqsb[:, p:p + 1])

                    def c_v0():
                        vt2 = vpool.tile([128, 16, 260], BF16, tag="v",
                                         name="va")
                        ones_ap = vt2.rearrange(
                            "p s (j c) -> p s j c", c=65)[:, :, :, 64:65]
                        nc.vector.memset(ones_ap, 1.0)
                        v_tiles[p] = (vt2, 0)
                        v_tiles[p + 1] = (vt2, 2)
                        st_["vt"] = vt2
                        _v_quarter(0)

                    def _v_quarter(g):
                        wv_sb, vt2 = st_["wv"], st_["vt"]
                        for st in range(g * 4, g * 4 + 4):
                            ps = ssp.tile([128, SQ], F32, tag="ss",
                                          name="ps_v")
                            for i in range(8):
                                nc.tensor.matmul(
                                    ps[:, 0:256],
                                    d16[:, i, st * 128:(st + 1) * 128],
                                    wv_sb[:, i, :],
                                    start=(i == 0), stop=(i == 7))
                            dst = vt2[:, st, :].rearrange(
                                "p (j c) -> p j c", c=65)
                            nc.vector.tensor_copy(
                                out=dst[:, :, 0:64],
                                in_=ps[:, 0:256].rearrange(
                                    "p (j c) -> p j c", c=64))

                    nop = lambda: None
                    if p % 2 == 0:
                        return dmas, [
                            lambda: c_k([0, 1]), lambda: c_k([2, 3]),
                            lambda: c_q([0, 1]), lambda: c_q([2, 3]),
                            c_v0, lambda: _v_quarter(1),
                            lambda: _v_quarter(2), lambda: _v_quarter(3)]
                    return dmas, [
                        lambda: c_k([0, 1]), lambda: c_k([2, 3]),
                        lambda: c_q([0, 1]), lambda: c_q([2, 3]),
                        nop, nop, nop, nop]

                def attention_section(p, qh2, h):
                    kt = k_tiles[p % 2]
                    qt = q_tiles[p % 2]
                    vt, jb = v_tiles[p]
                    q0 = qh2 * SQ
                    cps = cpsp.tile([65, SQ], F32, tag="cps", name="cps")
                    for i in range(16):
                        ss = ssp.tile([128, SQ], F32, tag="ss", name="ss")
                        for qh in range(2):
                            nc.tensor.matmul(
                                ss[:, qh * 512:(qh + 1) * 512],
                                kt[h * 64:(h + 1) * 64,
                                   i * 128:(i + 1) * 128],
                                qt[h * 64:(h + 1) * 64,
                                   q0 + qh * 512:q0 + (qh + 1) * 512],
                                start=True, stop=True)
                        et = epool.tile([128, SQ], BF16, tag="e", name="et")
                        nc.scalar.activation(out=et, in_=ss, func=AF.Exp)
                        nc.vector.tensor_mul(et, et, msb[:, i, q0:q0 + SQ])
                        for qh in range(2):
                            nc.tensor.matmul(
                                cps[0:65, qh * 512:(qh + 1) * 512],
                                vt[:, i, (jb + h) * 65:(jb + h + 1) * 65],
                                et[:, qh * 512:(qh + 1) * 512],
                                start=(i == 0), stop=(i == 15))
                    rec = rpool.tile([1, SQ], F32, tag="r", name="rec")
                    nc.vector.reciprocal(rec, cps[64:65, :])
                    bc = bcp.tile([64, SQ], F32, tag="bc", name="bc")
                    nc.gpsimd.partition_broadcast(bc, rec, channels=64)
                    nc.vector.tensor_tensor(
                        out=ctxT[p][h * 64:(h + 1) * 64, q0:q0 + SQ],
                        in0=cps[0:64, :], in1=bc, op=ALU.mult)

                dma0, comp0 = emit_proj(0)
                dma0()
                if rep == 0:
                    emit_data_dmas()
                for thunk in comp0:
                    thunk()
                wo_tiles = {}
                for p in range(NPL):
                    if p + 1 < NPL:
                        dman, compn = emit_proj(p + 1)
                        dman()
                    else:
                        for dh in range(2):
                            for pp_ in range(NPL):
                                wo_sb = wop.tile([128, 512], BF16,
                                                 tag=f"wo{dh}_{pp_}",
                                                 name="wo_sb")
                                nc.sync.dma_start(
                                    out=wo_sb,
                                    in_=woT[pp_ * 128:(pp_ + 1) * 128,
                                            dh * 512:(dh + 1) * 512])
                                wo_tiles[(dh, pp_)] = wo_sb
                        compn = None
                    si = 0
                    for qh2 in range(2):
                        for h in range(2):
                            attention_section(p, qh2, h)
                            if compn is not None:
                                compn[si]()
                                compn[si + 1]()
                            si += 2
                    v_tiles.pop(p)

            # ---------------- output projection --------------------------
            with ExitStack() as octx:
                opool = octx.enter_context(tc.tile_pool(name="ost", bufs=6))
                pso = octx.enter_context(
                    tc.tile_pool(name="pso", bufs=2, space="PSUM"))
                for g in range(4):
                    for dh in range(2):
                        pso_t = [pso.tile([128, 512], F32, tag=f"o{qt_}",
                                          name=f"pso{qt_}")
                                 for qt_ in range(4)]
                        for p in range(NPL):
                            for qt_ in range(4):
                                qa = g * 512 + qt_ * 128
                                nc.tensor.matmul(
                                    pso_t[qt_],
                                    ctxT[p][:, qa:qa + 128],
                                    wo_tiles[(dh, p)],
                                    start=(p == 0), stop=(p == NPL - 1))
                        for qt_ in range(4):
                            qa = g * 512 + qt_ * 128
                            ot = opool.tile([128, 512], BF16, tag="ot",
                                            name="ot")
                            if qt_ % 2 == 0:
                                nc.vector.tensor_copy(ot, pso_t[qt_])
                            else:
                                nc.scalar.copy(ot, pso_t[qt_])
                            nc.sync.dma_start(
                                out=out[qa:qa + 128,
                                        dh * 512:(dh + 1) * 512],
                                in_=ot)
            rctx.close()

    nc.finalize()
    return nc


def _get_nc(reps=1):
    key = f"nc{reps}"
    if key not in _CACHE:
        _CACHE[key] = _build(reps)
    return _CACHE[key]


def _prep_inputs(data, mask, Wq, bq, Wk, bk, Wv, bv, Wo, bo):
    bf = ml_dtypes.bfloat16
    data = np.asarray(data, dtype=np.float32)
    mask = np.asarray(mask)

    def wprep(W, blk, scale=1.0):
        WT = np.asarray(W, np.float32).T * scale
        nb = D // blk
        r = WT.reshape(8, 128, nb, blk).transpose(1, 2, 0, 3)
        return np.ascontiguousarray(r.astype(bf))

    wq_all = wprep(Wq, 128, scale=0.125)    # [128, 8, 8, 128]
    wk_all = wprep(Wk, 128)
    wv_all = wprep(Wv, 256)                 # [128, 4, 8, 256]
    WoT = np.asarray(Wo, np.float32).T
    bq_all = np.ascontiguousarray((0.125 * np.asarray(bq, np.float32))
                                  .reshape(8, 128).T)
    bk_all = np.ascontiguousarray(np.asarray(bk, np.float32)
                                  .reshape(8, 128).T)
    boe0 = (np.asarray(bo, np.float32)
            + np.asarray(Wo, np.float32) @ np.asarray(bv, np.float32))
    global _HOST_BIAS
    _HOST_BIAS = boe0

    d16_b, mT_b = [], []
    for b in range(B):
        dT = data[b].T
        d16_b.append(np.ascontiguousarray(
            dT.reshape(8, 128, S).transpose(1, 0, 2).astype(bf)))
        keep = ~mask[b]
        mT_b.append(np.ascontiguousarray(keep.T.astype(bf)))

    in_maps = []
    for c in range(8):
        b, hb = divmod(c, 2)
        in_maps.append({
            "d16": d16_b[b], "maskT": mT_b[b],
            "wq16": np.ascontiguousarray(wq_all[:, hb * 4:hb * 4 + 4]),
            "wk16": np.ascontiguousarray(wk_all[:, hb * 4:hb * 4 + 4]),
            "wv16": np.ascontiguousarray(wv_all[:, hb * 2:hb * 2 + 2]),
            "woT": np.ascontiguousarray(
                WoT[hb * 512:(hb + 1) * 512, :].astype(bf)),
            "bq2": np.ascontiguousarray(bq_all[:, hb * 4:hb * 4 + 4]),
            "bk2": np.ascontiguousarray(bk_all[:, hb * 4:hb * 4 + 4]),
        })
    return in_maps


_HOST_BIAS = None


def _gather(res):
    out = np.empty((B, S, D), np.float32)
    for b in range(B):
        out[b] = (res[2 * b]["out"].astype(np.float32)
                  + res[2 * b + 1]["out"].astype(np.float32))
    out += _HOST_BIAS
    return out


def kernel(**inputs):
    in_maps = _prep_inputs(**inputs)
    nc = _get_nc()
    res = run_bass_kernel_spmd(nc, in_maps, list(range(8))).results
    return _gather(res)

